# revision 11
# baseline (speedup 1.0000x reference)
"""Trainium2 Bass kernel for the DVBundle neuron-update step.

Reference computation (per full problem, fp32):
    I[n,k]   = sum_i w[n,i,k] * x[i,k]                     (n=4096, i=8192, k=4)
    dv[n]    = (I[n,0] - I[n,1] + I[n,2] - I[n,3] - v[n]) / TAU_V * DT
    reg[n]   = r[n] * act_d(v[n]) * dv[n]
    w_new    = w * (1 - reg[n]*I[n,k]/TAU_W) + (reg[n]*ALPHA/TAU_W) * x[i,k]
    r_new[n] = (v[n] > 0) * tanh(v[n])
    v_new[n] = v[n] + dv[n]

Sharding: w, v, r are split along the neuron axis across 8 NeuronCores
(512 neurons per core); x is replicated.  No cross-core communication.

Per-core kernel (SPMD, identical NEFF on every core):
  Setup:   x is replicated across all 128 partitions once (16 resident
           [128, 2048] tiles, built with stride-0 broadcast DMA reads).
  Phase 1: stream w tiles [128n x (512i x 4k)]; product on the Vector engine
           (contiguous tensor_tensor) + per-k strided tensor_reduce into an
           I accumulator.
  Finalize: tiny per-partition ops producing dv, reg, v_new, r_new, plus the
           per-(n,k) scale s = 1 - reg*I/TAU_W and per-n coefficient
           c = reg*ALPHA/TAU_W.
  Phase 2: re-stream w tiles; w_new = (w * s_bcast) then += c * x via one
           contiguous tensor_tensor + one fused scalar_tensor_tensor.

`repeats > 1` wraps the whole body in an on-device For loop — used only for
timing (amortizes the ~90ms axon dispatch overhead); the graded kernel()
path uses repeats=1.
"""

import os
import sys

for _p in ("/root/.axon_site", "/root/.axon_site/_ro/trn_rl_repo",
           "/root/.axon_site/_ro/pypackages", "/opt/trn_rl_repo"):
    if os.path.isdir(_p) and _p not in sys.path:
        sys.path.append(_p)

from contextlib import ExitStack, nullcontext

import numpy as np

import concourse.bacc as bacc
import concourse.tile as tile
from concourse import mybir
from concourse.bass_utils import run_bass_kernel_spmd

# Problem shape (hardcoded per spec)
N_CORES = 8
N, NI, K = 4096, 8192, 4
NL = N // N_CORES          # 512 neurons per core
P = 128                    # SBUF partitions
NT = NL // P               # 4 neuron tiles per core
CH = 512                   # i-chunk size
NCH = NI // CH             # 16 chunks
F = CH * K                 # 2048 free elements per w tile

# Module constants
DT = 0.01
TAU_V = 0.2
TAU_W = 50.0
ALPHA = 0.5
DV_SCALE = DT / TAU_V              # 0.05
NEG_INV_TAU_W = -1.0 / TAU_W       # -0.02
C_SCALE = ALPHA / TAU_W            # 0.01

_F32 = mybir.dt.float32
_MULT = mybir.AluOpType.mult
_ADD = mybir.AluOpType.add
_SUB = mybir.AluOpType.subtract
_IS_GT = mybir.AluOpType.is_gt


def _build_bass(repeats=1):
    nc = bacc.Bacc("TRN2", debug=False)

    w_in = nc.dram_tensor("w", [NL, NI, K], _F32, kind="ExternalInput")
    x_in = nc.dram_tensor("x", [NI, K], _F32, kind="ExternalInput")
    v_in = nc.dram_tensor("v", [NL], _F32, kind="ExternalInput")
    r_in = nc.dram_tensor("r", [NL], _F32, kind="ExternalInput")
    w_out = nc.dram_tensor("w_new", [NL, NI, K], _F32, kind="ExternalOutput")
    v_out = nc.dram_tensor("v_new", [NL], _F32, kind="ExternalOutput")
    r_out = nc.dram_tensor("r_new", [NL], _F32, kind="ExternalOutput")

    with tile.TileContext(nc) as tc, ExitStack() as ctx:
        consts = ctx.enter_context(tc.tile_pool(name="consts", bufs=1))
        small = ctx.enter_context(tc.tile_pool(name="small", bufs=1))
        xa_pool = ctx.enter_context(tc.tile_pool(name="xa", bufs=NCH))

        # x broadcast across all partitions, one resident tile per i-chunk
        xa_tiles = []
        for ic in range(NCH):
            xa_t = xa_pool.tile([P, F], _F32)
            nc.sync.dma_start(
                out=xa_t[:],
                in_=x_in[ic * CH:(ic + 1) * CH, :]
                .rearrange("i k -> (i k)")
                .unsqueeze(0)
                .broadcast_to([P, F]),
            )
            xa_tiles.append(xa_t)

        # v, r as [P, NT]: column t holds neurons [t*128, (t+1)*128)
        vt = consts.tile([P, NT], _F32)
        rt = consts.tile([P, NT], _F32)
        nc.sync.dma_start(out=vt[:], in_=v_in[:].rearrange("(t p) -> p t", p=P))
        nc.sync.dma_start(out=rt[:], in_=r_in[:].rearrange("(t p) -> p t", p=P))

        # I accumulator: per (nt, ic, k) partial dot products
        iacc = small.tile([P, NT * NCH * K], _F32)

        # finalize scratch (allocated once, rewritten per repeat)
        i4 = small.tile([P, NT * K], _F32)      # I[n,k]
        s4 = small.tile([P, NT * K], _F32)      # 1 - reg*I/TAU_W
        c_col = small.tile([P, NT], _F32)       # reg*ALPHA/TAU_W
        vnew = small.tile([P, NT], _F32)
        rnew = small.tile([P, NT], _F32)
        tmp = small.tile([P, 8 * NT], _F32)     # scratch columns

        # Streaming pools shared by both phases (avoids pool release/realloc
        # dependency pile-ups that overflow per-instruction sync wait slots).
        w_pool = ctx.enter_context(tc.tile_pool(name="wstream", bufs=3))
        scr_pool = ctx.enter_context(tc.tile_pool(name="scr", bufs=1))
        wn_pool = ctx.enter_context(tc.tile_pool(name="wn", bufs=3))

        def w_dram(t, c, handle):
            return handle[
                t * P:(t + 1) * P, c * CH:(c + 1) * CH, :
            ].rearrange("p i k -> p (i k)")

        def body():
            # ---------------- Phase 1: I[n,k] accumulation ----------------
            # (tensor_tensor_reduce faults the exec unit on this runtime even
            # for contiguous APs; use a contiguous multiply plus a strided
            # per-k tensor_reduce instead.)
            for ic in range(NCH):
                for nt in range(NT):
                    wt = w_pool.tile([P, F], _F32, tag="w_stream")
                    nc.sync.dma_start(out=wt[:], in_=w_dram(nt, ic, w_in))
                    scr = scr_pool.tile([P, F], _F32, tag="scr")
                    nc.vector.tensor_tensor(
                        out=scr[:], in0=wt[:], in1=xa_tiles[ic][:], op=_MULT)
                    col = (nt * NCH + ic) * K
                    nc.vector.tensor_reduce(
                        out=iacc[:, col:col + K],
                        in_=scr[:].rearrange("p (i k) -> p k i", k=K),
                        axis=mybir.AxisListType.X,
                        op=_ADD,
                    )

            # ---------------- Finalize: per-neuron scalars ----------------
            for nt in range(NT):
                i4_nt = i4[:, nt * K:(nt + 1) * K]
                # I[n,k] = sum over ic of partials
                nc.vector.tensor_reduce(
                    out=i4_nt,
                    in_=iacc[:, nt * NCH * K:(nt + 1) * NCH * K].rearrange(
                        "p (ic k) -> p k ic", k=K),
                    axis=mybir.AxisListType.X,
                    op=_ADD,
                )
                b = 8 * nt  # scratch column base for this neuron tile
                t0 = tmp[:, b + 0:b + 1]   # I0 - I1
                t1 = tmp[:, b + 1:b + 2]   # I2 - I3, then (...) - v
                dv = tmp[:, b + 2:b + 3]
                th = tmp[:, b + 3:b + 4]   # tanh(v)
                mask = tmp[:, b + 4:b + 5]
                actd = tmp[:, b + 5:b + 6]
                reg = tmp[:, b + 6:b + 7]
                su = s4[:, nt * K:(nt + 1) * K]
                v_nt = vt[:, nt:nt + 1]
                r_nt = rt[:, nt:nt + 1]

                nc.vector.tensor_tensor(
                    out=t0, in0=i4_nt[:, 0:1], in1=i4_nt[:, 1:2], op=_SUB)
                nc.vector.tensor_tensor(
                    out=t1, in0=i4_nt[:, 2:3], in1=i4_nt[:, 3:4], op=_SUB)
                nc.vector.tensor_tensor(out=t0, in0=t0, in1=t1, op=_ADD)
                nc.vector.tensor_tensor(out=t1, in0=t0, in1=v_nt, op=_SUB)
                nc.vector.tensor_scalar(
                    out=dv, in0=t1, scalar1=DV_SCALE, scalar2=None, op0=_MULT)

                nc.scalar.activation(
                    out=th, in_=v_nt, func=mybir.ActivationFunctionType.Tanh)
                nc.vector.tensor_scalar(
                    out=mask, in0=v_nt, scalar1=0.0, scalar2=None, op0=_IS_GT)
                # r_new = mask * tanh(v)
                nc.vector.tensor_tensor(
                    out=rnew[:, nt:nt + 1], in0=mask, in1=th, op=_MULT)
                # act_d = mask * (1 - tanh^2)
                nc.vector.tensor_tensor(out=actd, in0=th, in1=th, op=_MULT)
                nc.vector.tensor_scalar(
                    out=actd, in0=actd, scalar1=-1.0, scalar2=1.0,
                    op0=_MULT, op1=_ADD)
                nc.vector.tensor_tensor(out=actd, in0=actd, in1=mask, op=_MULT)
                # reg = r * act_d * dv
                nc.vector.tensor_tensor(out=reg, in0=r_nt, in1=actd, op=_MULT)
                nc.vector.tensor_tensor(out=reg, in0=reg, in1=dv, op=_MULT)
                # v_new = v + dv
                nc.vector.tensor_tensor(
                    out=vnew[:, nt:nt + 1], in0=v_nt, in1=dv, op=_ADD)
                # s = 1 - reg*I/TAU_W  (fused: (I*reg)*(-1/TAU_W) + 1)
                nc.vector.tensor_scalar(
                    out=su, in0=i4_nt, scalar1=reg, scalar2=None, op0=_MULT)
                nc.vector.tensor_scalar(
                    out=su, in0=su, scalar1=NEG_INV_TAU_W, scalar2=1.0,
                    op0=_MULT, op1=_ADD)
                # c = reg * ALPHA/TAU_W
                nc.vector.tensor_scalar(
                    out=c_col[:, nt:nt + 1], in0=reg, scalar1=C_SCALE,
                    scalar2=None, op0=_MULT)

            nc.sync.dma_start(
                out=v_out[:].rearrange("(t p) -> p t", p=P), in_=vnew[:])
            nc.sync.dma_start(
                out=r_out[:].rearrange("(t p) -> p t", p=P), in_=rnew[:])

            # ---------------- Phase 2: w_new = s*w + c*x ----------------
            for ic in range(NCH):
                for nt in range(NT):
                    wt = w_pool.tile([P, F], _F32, tag="w_stream")
                    nc.sync.dma_start(out=wt[:], in_=w_dram(nt, ic, w_in))
                    wn = wn_pool.tile([P, F], _F32, tag="wn")
                    s_b = (s4[:, nt * K:(nt + 1) * K]
                           .unsqueeze(1).broadcast_to([P, CH, K]))
                    # w_new = w * s[n,k]
                    nc.vector.tensor_tensor(
                        out=wn[:].rearrange("p (i k) -> p i k", k=K),
                        in0=wt[:].rearrange("p (i k) -> p i k", k=K),
                        in1=s_b,
                        op=_MULT,
                    )
                    # w_new += c[n] * x  (fused, in-place)
                    nc.vector.scalar_tensor_tensor(
                        out=wn[:],
                        in0=xa_tiles[ic][:],
                        scalar=c_col[:, nt:nt + 1],
                        in1=wn[:],
                        op0=_MULT,
                        op1=_ADD,
                    )
                    nc.sync.dma_start(out=w_dram(nt, ic, w_out), in_=wn[:])

        if repeats > 1:
            loop = tc.For_i(
                0, repeats, 1,
                hint_engines=(mybir.EngineType.DVE, mybir.EngineType.SP),
            )
        else:
            loop = nullcontext()
        with loop:
            body()

    # Run the bacc passes (register allocation, nop fusion, event-semaphore
    # generation that splits multi-wait instructions) before serialization.
    nc.finalize()
    return nc


_NC_CACHE = {}


def _get_nc(repeats=1):
    if repeats not in _NC_CACHE:
        _NC_CACHE[repeats] = _build_bass(repeats)
    return _NC_CACHE[repeats]


def run(w, x, v, r, trace=False, repeats=1, **spmd_kwargs):
    """Shard inputs, run the SPMD kernel on 8 cores, gather full outputs.

    Returns ((v_new, r_new, w_new), BassKernelResults).
    """
    w = np.ascontiguousarray(w, dtype=np.float32)
    x = np.ascontiguousarray(x, dtype=np.float32)
    v = np.ascontiguousarray(v, dtype=np.float32)
    r = np.ascontiguousarray(r, dtype=np.float32)

    nc = _get_nc(repeats)
    core_ids = list(range(N_CORES))
    in_maps = [
        {
            "w": w[c * NL:(c + 1) * NL],
            "x": x,
            "v": v[c * NL:(c + 1) * NL],
            "r": r[c * NL:(c + 1) * NL],
        }
        for c in core_ids
    ]
    res = run_bass_kernel_spmd(nc, in_maps, core_ids, trace=trace,
                               **spmd_kwargs)
    v_new = np.concatenate([m["v_new"] for m in res.results])
    r_new = np.concatenate([m["r_new"] for m in res.results])
    w_new = np.concatenate([m["w_new"] for m in res.results])
    return (v_new, r_new, w_new), res


def kernel(w, x, v, r):
    (v_new, r_new, w_new), _ = run(w, x, v, r)
    return v_new, r_new, w_new
